# revision 36
# baseline (speedup 1.0000x reference)
"""Trainium2 Bass kernel for a spatial-attention module (nn_Atten).

Math (per batch b):
  cv[r,a]   = sum_h V[b,r,h]   * Wv[a,h]
  gh[t,a]   = sum_h h_t[b,t,h] * Wg[a,h]
  z[t,r]    = sum_a wh[a] * tanh(gh[t,a] + cv[r,a])
  alpha     = softmax_r(z)                       # output [B,T,R]
  cs[t,a]   = sum_h s_t[b,t,h] * Ws[a,h] + gh[t,a]
  z_ext[t]  = sum_a wh[a] * tanh(cs[t,a])
  beta      = softmax([z, z_ext])[..., -1]       # output [B,T,1]
  c_hat     = beta*s_t + (1-beta)*(alpha @ V)    # output [B,T,H]

Sharding: pure data parallel, batch dim 64 -> 8 cores x 8 batches.

Device dataflow per (b, t-tile of 128):
  PE   : content[t, a*49+r] = sum_c ghx[c,t]*REP[c,col]  (C=50: 49 ghT rows +
         a ones row carrying the flattened cv vector) -> PSUM, 5 matmuls N<=490.
  ACT  : tanh PSUM -> SBUF fp16 mega tile, layout col = a*50 + r (pitch 50 keeps
         every a-block 4-byte aligned for DVE 2x mode).
  DVE  : y = tanh * wh_rep (fp16 2x), then pairwise a-block add tree -> z.
  ACT  : exp over [z | z_ext]; DVE: sums/recip -> alpha, beta.
  PE   : alphasT = transpose((1-beta)*alpha); c' = alphasT.T @ V.
  DVE  : c_hat = (s_t * beta) + c'   (single fused scalar_tensor_tensor).
"""

import numpy as np

B, R, T, H, A = 64, 49, 512, 512, 49
NCORES = 8
BPC = B // NCORES            # batches per core
NTT = T // 128               # t-tiles per batch
PITCH = 50                   # a-block pitch in the tanh mega tile (49 data + 1 pad)
TTW = A * PITCH              # 2450 cols per t-tile in mega
CW = A * R                   # 2401 compact content cols
CHUNK = 490                  # content matmul free-dim chunk (10 a-blocks)

_CACHE = {}


def build_nc():
    import concourse.bass as bass
    import concourse.mybir as mybir
    from concourse import bacc, tile
    from concourse.tile_rust import add_dep_helper as tile_add_dep
    from contextlib import ExitStack

    dt = mybir.dt
    AF = mybir.ActivationFunctionType
    OP = mybir.AluOpType
    CONTENT_DT = dt.float16     # ghx / REP / content matmul ins
    TANH_DT = dt.float16         # tanh_out / wh_rep / tree

    nc = bacc.Bacc("TRN2", target_bir_lowering=False, debug=False)

    # ---- DRAM tensors (per-core shard; host ships partition-major layouts) ----
    hT = nc.dram_tensor("hT", [BPC, 128, 4, T], dt.float16, kind="ExternalInput")
    sT = nc.dram_tensor("sT", [BPC, 128, 4, T], dt.float16, kind="ExternalInput")
    st = nc.dram_tensor("st", [BPC, 128, NTT, H], dt.float16, kind="ExternalInput")
    v = nc.dram_tensor("v", [BPC, R, H], dt.float32, kind="ExternalInput")
    vT = nc.dram_tensor("vT", [BPC, 128, 4, R], dt.float16, kind="ExternalInput")
    wgT = nc.dram_tensor("wgT", [128, 4, A], dt.float16, kind="ExternalInput")
    wsT = nc.dram_tensor("wsT", [128, 4, A], dt.float16, kind="ExternalInput")
    wvT = nc.dram_tensor("wvT", [128, 4, A], dt.float16, kind="ExternalInput")
    repc = nc.dram_tensor("repc", [A, CW], CONTENT_DT, kind="ExternalInput")
    cvs = nc.dram_tensor("cvs", [BPC, 1, CW], CONTENT_DT, kind="Internal")
    whrep = nc.dram_tensor("whrep", [128, TTW], TANH_DT, kind="ExternalInput")
    wh49 = nc.dram_tensor("wh49", [128, NTT * A], TANH_DT, kind="ExternalInput")
    ident = nc.dram_tensor("ident", [128, 128], dt.float32, kind="ExternalInput")
    ones1 = nc.dram_tensor("ones1", [1, T], CONTENT_DT, kind="ExternalInput")

    chat_d = nc.dram_tensor("chat", [BPC, NTT, 128, H], dt.float32, kind="ExternalOutput")
    alpha_d = nc.dram_tensor("alpha", [BPC, 128, NTT, R], dt.float32, kind="ExternalOutput")
    beta_d = nc.dram_tensor("beta", [BPC, 128, NTT], dt.float32, kind="ExternalOutput")

    ctx = ExitStack()
    with tile.TileContext(nc) as tc, ctx:
        pool = ctx.enter_context(tc.tile_pool(name="main", bufs=1))
        cpool = ctx.enter_context(tc.tile_pool(name="const", bufs=1))
        ppool = ctx.enter_context(tc.tile_pool(name="ps", space="PSUM", bufs=1))

        # ---- constants (loaded once, distinct tags so nothing aliases) ----
        whrep_sb = cpool.tile([128, TTW], TANH_DT, tag="whrep")
        nc.gpsimd.dma_start(whrep_sb[:], whrep[:])
        wh49_sb = cpool.tile([128, NTT * A], TANH_DT, tag="wh49")
        nc.gpsimd.dma_start(wh49_sb[:], wh49[:])
        ident_sb = cpool.tile([128, 128], dt.float32, tag="ident")
        nc.gpsimd.dma_start(ident_sb[:], ident[:])
        wgT_sb = cpool.tile([128, 4 * A], dt.float16, tag="wgT")
        nc.gpsimd.dma_start(wgT_sb[:].rearrange("p (c a) -> p c a", a=A), wgT[:])
        wsT_sb = cpool.tile([128, 4 * A], dt.float16, tag="wsT")
        nc.gpsimd.dma_start(wsT_sb[:].rearrange("p (c a) -> p c a", a=A), wsT[:])
        wvT_sb = cpool.tile([128, 4 * A], dt.float16, tag="wvT")
        nc.gpsimd.dma_start(wvT_sb[:].rearrange("p (c a) -> p c a", a=A), wvT[:])
        # two parity REP tiles: rows 0..48 constant, row 49 = per-batch cv flat
        reps = []
        for par in range(2):
            rt = cpool.tile([A + 1, CW], CONTENT_DT, name=f"rep{par}", tag=f"rep{par}")
            nc.gpsimd.dma_start(rt[0:A, :], repc[:])
            reps.append(rt)

        def stage_A(b):
            """inputs, cvT->REP row, ghT/ghx, cs+tanh, content+tanh. PE/ACT/DMA."""
            rep_sb = reps[b % 2]
            vT_sb = pool.tile([128, 4 * R], dt.float16, tag="vT", bufs=3, name=f"vT{b}")
            nc.sync.dma_start(vT_sb[:].rearrange("p (c r) -> p c r", r=R), vT[b])
            v_sb = pool.tile([R, H], dt.float32, tag="v", bufs=3, name=f"v{b}")
            nc.sync.dma_start(v_sb[:], v[b])
            hT_sb = pool.tile([128, 4 * T], dt.float16, tag="hT", bufs=3, name=f"hT{b}")
            for c in range(4):
                nc.sync.dma_start(hT_sb[:, c * T:(c + 1) * T], hT[b, :, c])
            sT_sb = pool.tile([128, 4 * T], dt.float16, tag="sT", bufs=3, name=f"sT{b}")
            nc.sync.dma_start(sT_sb[:].rearrange("p (c t) -> p c t", t=T), sT[b])
            st_sb = pool.tile([128, NTT * H], dt.float16, tag="st", bufs=3, name=f"st{b}")
            nc.gpsimd.dma_start(st_sb[:].rearrange("p (c t) -> p c t", t=H), st[b])

            cvT_ps = ppool.tile([A, R], dt.float32, tag="ghTcs", bufs=1, name=f"cvT{b}")
            for c in range(4):
                nc.tensor.matmul(
                    out=cvT_ps[:],
                    lhsT=wvT_sb[:, c * A:(c + 1) * A],
                    rhs=vT_sb[:, c * R:(c + 1) * R],
                    start=(c == 0), stop=(c == 3),
                )
            cvf_sb = pool.tile([A, R], CONTENT_DT, tag="cvf", bufs=3, name=f"cvf{b}")
            nc.scalar.add(out=cvf_sb[:], in_=cvT_ps[:], add=0.0)
            wdma = nc.sync.dma_start(
                cvs[b].rearrange("one (a r) -> (one a) r", r=R), cvf_sb[:]
            )
            rdma = nc.sync.dma_start(rep_sb[A:A + 1, :], cvs[b])
            tile_add_dep(rdma.ins, wdma.ins, reason="cv flatten RAW through dram")

            ghT_ps = ppool.tile([A, T], dt.float32, tag="ghTcs", bufs=1, name=f"ghT{b}")
            for c in range(4):
                nc.tensor.matmul(
                    out=ghT_ps[:],
                    lhsT=wgT_sb[:, c * A:(c + 1) * A],
                    rhs=hT_sb[:, c * T:(c + 1) * T],
                    start=(c == 0), stop=(c == 3),
                )
            ghx_sb = pool.tile([A + 1, T], CONTENT_DT, tag="ghx", bufs=3, name=f"ghx{b}")
            nc.scalar.add(out=ghx_sb[0:A, :], in_=ghT_ps[:], add=0.0)
            nc.sync.dma_start(ghx_sb[A:A + 1, :], ones1[:])

            mega = pool.tile([128, NTT * TTW], TANH_DT, tag="mega", bufs=3, name=f"mega{b}")
            mv = mega[:].rearrange("p (t a r) -> p t a r", a=A, r=PITCH)
            nc.gpsimd.memset(mv[:, :, :, 49], 0.0)
            tcs = pool.tile([128, NTT * A], TANH_DT, tag="tcs", bufs=3, name=f"tcs{b}")
            for tt in range(NTT):
                cs_ps = ppool.tile([128, A], dt.float32, tag="ghTcs", bufs=1, name=f"cs{b}_{tt}")
                for c in range(4):
                    nc.tensor.matmul(
                        out=cs_ps[:],
                        lhsT=sT_sb[:, c * T + tt * 128: c * T + (tt + 1) * 128],
                        rhs=wsT_sb[:, c * A:(c + 1) * A],
                        start=(c == 0), stop=False,
                    )
                for c in range(4):
                    nc.tensor.matmul(
                        out=cs_ps[:],
                        lhsT=hT_sb[:, c * T + tt * 128: c * T + (tt + 1) * 128],
                        rhs=wgT_sb[:, c * A:(c + 1) * A],
                        start=False, stop=(c == 3),
                    )
                nc.scalar.activation(out=tcs[:, tt * A:(tt + 1) * A], in_=cs_ps[:], func=AF.Tanh)
                pa = ppool.tile([128, 1024], dt.float32, tag="cA", bufs=1, name=f"cA{b}_{tt}")
                pb = ppool.tile([128, 1024], dt.float32, tag="cB", bufs=1, name=f"cB{b}_{tt}")
                pc_ = ppool.tile([128, 512], dt.float32, tag="cC", bufs=1, name=f"cC{b}_{tt}")
                lhs = ghx_sb[:, tt * 128:(tt + 1) * 128]
                dests = [pa[:, 0:490], pa[:, 512:1002], pb[:, 0:490], pb[:, 512:1002],
                         pc_[:, 0:441]]
                for k in range(5):
                    n = CHUNK if k < 4 else CW - 4 * CHUNK
                    nc.tensor.matmul(
                        out=dests[k], lhsT=lhs,
                        rhs=rep_sb[:, k * CHUNK:k * CHUNK + n],
                        start=True, stop=True,
                    )
                base = tt * TTW
                for pt, nblk, off in ((pa, 10, 0), (pb, 10, 1000), (pc_, 9, 2000)):
                    if nblk == 10:
                        srcv = pt[:].rearrange("p (c x) -> p c x", x=512)[:, :, 0:490]
                        srcv = srcv.rearrange("p c (a r) -> p c a r", r=R)
                        dstv = mega[:, base + off: base + off + 1000]
                        dstv = dstv.rearrange("p (c a r) -> p c a r", c=2, r=PITCH)[:, :, :, 0:R]
                    else:
                        srcv = pt[:, 0:441].rearrange("p (a r) -> p a r", r=R)
                        dstv = mega[:, base + off: base + off + 450]
                        dstv = dstv.rearrange("p (a r) -> p a r", r=PITCH)[:, :, 0:R]
                    nc.scalar.activation(out=dstv, in_=srcv, func=AF.Tanh)
            return dict(st_sb=st_sb, v_sb=v_sb, mega=mega, mv=mv, tcs=tcs)

        def stage_B1(b, s):
            """wh multiply + pairwise tree + z_ext -> zm. DVE."""
            mega, mv, tcs = s["mega"], s["mv"], s["tcs"]
            mtv = mega[:].rearrange("p (t x) -> p t x", x=TTW)
            wbc = whrep_sb[:].broadcast(0, NTT) if hasattr(whrep_sb[:], "broadcast") else None
            if wbc is not None:
                nc.vector.tensor_tensor(out=mtv, in0=mtv, in1=wbc, op=OP.mult)
            else:
                for tt in range(NTT):
                    sl = mega[:, tt * TTW:(tt + 1) * TTW]
                    nc.vector.tensor_tensor(out=sl, in0=sl, in1=whrep_sb[:], op=OP.mult)
            t1 = pool.tile([128, NTT * 24 * PITCH], TANH_DT, tag="t1", name=f"t1_{b}")
            t1v = t1[:].rearrange("p (t a r) -> p t a r", a=24, r=PITCH)
            nc.vector.tensor_tensor(out=t1v, in0=mv[:, :, 0:24, :], in1=mv[:, :, 25:49, :], op=OP.add)
            t2 = pool.tile([128, NTT * 12 * PITCH], TANH_DT, tag="t2", name=f"t2_{b}")
            t2v = t2[:].rearrange("p (t a r) -> p t a r", a=12, r=PITCH)
            nc.vector.tensor_tensor(out=t2v, in0=t1v[:, :, 0:12, :], in1=t1v[:, :, 12:24, :], op=OP.add)
            t3 = pool.tile([128, NTT * 6 * PITCH], TANH_DT, tag="t3", name=f"t3_{b}")
            t3v = t3[:].rearrange("p (t a r) -> p t a r", a=6, r=PITCH)
            nc.vector.tensor_tensor(out=t3v, in0=t2v[:, :, 0:6, :], in1=t2v[:, :, 6:12, :], op=OP.add)
            t4 = pool.tile([128, NTT * 3 * PITCH], TANH_DT, tag="t4", name=f"t4_{b}")
            t4v = t4[:].rearrange("p (t a r) -> p t a r", a=3, r=PITCH)
            nc.vector.tensor_tensor(out=t4v, in0=t3v[:, :, 0:3, :], in1=t3v[:, :, 3:6, :], op=OP.add)
            t5 = pool.tile([128, NTT * PITCH], TANH_DT, tag="t5", name=f"t5_{b}")
            t5v = t5[:].rearrange("p (t r) -> p t r", r=PITCH)
            nc.vector.tensor_tensor(out=t5v, in0=t4v[:, :, 0, :], in1=t4v[:, :, 1, :], op=OP.add)
            t6 = pool.tile([128, NTT * PITCH], TANH_DT, tag="t6", name=f"t6_{b}")
            t6v = t6[:].rearrange("p (t r) -> p t r", r=PITCH)
            nc.vector.tensor_tensor(out=t6v, in0=t5v[:], in1=t4v[:, :, 2, :], op=OP.add)
            zm = pool.tile([128, NTT * 64], dt.float32, tag="zm", bufs=3, name=f"zm{b}")
            zv = zm[:].rearrange("p (t c) -> p t c", c=64)
            nc.vector.tensor_tensor(
                out=zv[:, :, 0:R], in0=t6v[:, :, 0:R], in1=mv[:, :, 24, 0:R], op=OP.add
            )
            nc.vector.tensor_tensor(out=tcs[:], in0=tcs[:], in1=wh49_sb[:], op=OP.mult)
            ysv = tcs[:].rearrange("p (t a) -> p t a", a=A)
            nc.vector.reduce_sum(out=zv[:, :, 49], in_=ysv, axis=mybir.AxisListType.X)
            s["zm"], s["zv"] = zm, zv

        def stage_B2(b, s):
            """exp + softmax + alpha/beta out. ACT(1) + small DVE."""
            zv = s["zv"]
            em = pool.tile([128, NTT * 64], dt.float32, tag="em", bufs=3, name=f"em{b}")
            ev = em[:].rearrange("p (t c) -> p t c", c=64)
            nc.scalar.activation(out=ev[:, :, 0:50], in_=zv[:, :, 0:50], func=AF.Exp)
            s50 = pool.tile([128, NTT], dt.float32, tag="s50", name=f"s50_{b}")
            nc.vector.reduce_sum(out=s50[:], in_=ev[:, :, 0:50], axis=mybir.AxisListType.X)
            s49 = pool.tile([128, NTT], dt.float32, tag="s49", name=f"s49_{b}")
            nc.vector.tensor_tensor(out=s49[:], in0=s50[:], in1=ev[:, :, 49], op=OP.subtract)
            r49 = pool.tile([128, NTT], dt.float32, tag="r49", name=f"r49_{b}")
            nc.vector.reciprocal(out=r49[:], in_=s49[:])
            r50 = pool.tile([128, NTT], dt.float32, tag="r50", name=f"r50_{b}")
            nc.vector.reciprocal(out=r50[:], in_=s50[:])
            beta_sb = pool.tile([128, NTT], dt.float32, tag="beta", bufs=3, name=f"beta{b}")
            nc.vector.tensor_tensor(out=beta_sb[:], in0=ev[:, :, 49], in1=r50[:], op=OP.mult)
            omb = pool.tile([128, NTT], dt.float32, tag="omb", name=f"omb{b}")
            nc.vector.tensor_scalar(
                out=omb[:], in0=beta_sb[:], scalar1=-1.0, scalar2=1.0, op0=OP.mult, op1=OP.add
            )
            nc.sync.dma_start(beta_d[b], beta_sb[:])
            alpha_sb = pool.tile([128, NTT * R], dt.float32, tag="alpha", bufs=3, name=f"al{b}")
            av = alpha_sb[:].rearrange("p (t r) -> p t r", r=R)
            for tt in range(NTT):
                nc.vector.tensor_scalar(
                    out=av[:, tt, :], in0=ev[:, tt, 0:R],
                    scalar1=r49[:, tt:tt + 1], scalar2=None, op0=OP.mult,
                )
            nc.sync.dma_start(alpha_d[b], av)
            s["av"], s["omb"], s["beta_sb"] = av, omb, beta_sb

        def stage_C(b, s):
            """c' = ((1-beta)*alpha) @ V; c_hat blend + store. PE + DVE."""
            av, omb, beta_sb = s["av"], s["omb"], s["beta_sb"]
            st_sb, v_sb = s["st_sb"], s["v_sb"]
            aT_sb = pool.tile([R, T], dt.float32, tag="aT", bufs=3, name=f"aT{b}")
            for tt in range(NTT):
                asc = pool.tile([128, R], dt.float32, tag="asc", bufs=3, name=f"asc{b}_{tt}")
                nc.vector.tensor_scalar(
                    out=asc[:], in0=av[:, tt, :],
                    scalar1=omb[:, tt:tt + 1], scalar2=None, op0=OP.mult,
                )
                aT_ps = ppool.tile([R, 128], dt.float32, tag="smallps", bufs=1, name=f"aT{b}_{tt}")
                nc.tensor.transpose(out=aT_ps[:], in_=asc[:], identity=ident_sb[:])
                nc.scalar.add(out=aT_sb[:, tt * 128:(tt + 1) * 128], in_=aT_ps[:], add=0.0)
            chat_sb = pool.tile([128, NTT * H], dt.float32, tag="chat", bufs=2, name=f"ch{b}")
            for tt in range(NTT):
                cps = ppool.tile([128, H], dt.float32, tag="cps", bufs=1, name=f"cps{b}_{tt}")
                nc.tensor.matmul(
                    out=cps[:], lhsT=aT_sb[:, tt * 128:(tt + 1) * 128], rhs=v_sb[:],
                    start=True, stop=True,
                )
                nc.vector.scalar_tensor_tensor(
                    out=chat_sb[:, tt * H:(tt + 1) * H], in0=st_sb[:, tt * H:(tt + 1) * H],
                    scalar=beta_sb[:, tt:tt + 1], in1=cps[:],
                    op0=OP.mult, op1=OP.add,
                )
            for tt in range(NTT):
                nc.gpsimd.dma_start(chat_d[b, tt], chat_sb[:, tt * H:(tt + 1) * H])

        # Software pipeline: per iteration emit softmax(b-1), c_hat(b-1),
        # then content/tanh(b), then the DVE reduction(b). Keeps every
        # engine's in-order queue free of long cross-stage stalls.
        states = {}
        for it in range(BPC + 1):
            if it >= 1:
                stage_B2(it - 1, states[it - 1])
            if it < BPC:
                states[it] = stage_A(it)
            if it >= 1:
                stage_C(it - 1, states[it - 1])
            if it < BPC:
                stage_B1(it, states[it])
            if it >= 1:
                del states[it - 1]

    nc.compile()
    return nc


def host_prep(V, h_t, s_t, Wv, Wg, Ws, Wh):
    f32 = np.float32
    fp16 = np.float16
    bf16 = np.float16  # content path is fp16 end to end

    V = np.asarray(V, f32)
    h_t = np.asarray(h_t, f32)
    s_t = np.asarray(s_t, f32)
    wh = np.asarray(Wh, f32)[0]

    repc = np.zeros((A, CW), f32)
    for a in range(A):
        repc[a, a * R:(a + 1) * R] = 1.0
    whrep = np.zeros((TTW,), f32)
    whrep.reshape(A, PITCH)[:, :R] = wh[:, None]

    def wT(w):  # [A,H] -> [128, 4, A] partition-major chunks of w.T
        return np.ascontiguousarray(
            np.asarray(w, f32).T.reshape(4, 128, A).transpose(1, 0, 2)
        ).astype(bf16)

    consts = {
        "repc": np.ascontiguousarray(repc, dtype=bf16),
        "whrep": np.ascontiguousarray(np.broadcast_to(whrep, (128, TTW)), dtype=fp16),
        "wh49": np.ascontiguousarray(
            np.broadcast_to(np.tile(wh, NTT), (128, NTT * A)), dtype=fp16),
        "ident": np.eye(128, dtype=f32),
        "ones1": np.ones((1, T), dtype=bf16),
        "wgT": wT(Wg), "wsT": wT(Ws), "wvT": wT(Wv),
    }

    in_maps = []
    for core in range(NCORES):
        sl = slice(core * BPC, (core + 1) * BPC)
        hb, sb, vb = h_t[sl], s_t[sl], V[sl]
        m = dict(consts)
        # [BPC,T,H] -> transpose -> [BPC,H,T] -> [BPC,4,128,T] -> [BPC,128,4,T]
        m["hT"] = np.ascontiguousarray(
            hb.transpose(0, 2, 1).reshape(BPC, 4, 128, T).transpose(0, 2, 1, 3)
        ).astype(bf16)
        m["sT"] = np.ascontiguousarray(
            sb.transpose(0, 2, 1).reshape(BPC, 4, 128, T).transpose(0, 2, 1, 3)
        ).astype(bf16)
        m["st"] = np.ascontiguousarray(
            sb.reshape(BPC, NTT, 128, H).transpose(0, 2, 1, 3)
        ).astype(fp16)
        m["v"] = np.ascontiguousarray(vb)
        m["vT"] = np.ascontiguousarray(
            vb.transpose(0, 2, 1).reshape(BPC, 4, 128, R).transpose(0, 2, 1, 3)
        ).astype(bf16)
        in_maps.append(m)
    return in_maps


def gather(results):
    chat = np.concatenate([np.asarray(r["chat"]).reshape(BPC, T, H) for r in results], axis=0)
    alpha = np.concatenate(
        [np.asarray(r["alpha"]).transpose(0, 2, 1, 3).reshape(BPC, T, R) for r in results], axis=0
    )
    beta = np.concatenate(
        [np.asarray(r["beta"]).transpose(0, 2, 1).reshape(BPC, T, 1) for r in results], axis=0
    )
    return chat, alpha, beta


def kernel(V, h_t, s_t, Wv, Wg, Ws, Wh):
    from concourse.bass_utils import run_bass_kernel_spmd

    if "nc" not in _CACHE:
        _CACHE["nc"] = build_nc()
    nc = _CACHE["nc"]
    in_maps = host_prep(V, h_t, s_t, Wv, Wg, Ws, Wh)
    res = run_bass_kernel_spmd(nc, in_maps, core_ids=list(range(NCORES)))
    return gather(res.results)


# revision 37
# speedup vs baseline: 1.0199x; 1.0199x over previous
"""Trainium2 Bass kernel for a spatial-attention module (nn_Atten).

Math (per batch b):
  cv[r,a]   = sum_h V[b,r,h]   * Wv[a,h]
  gh[t,a]   = sum_h h_t[b,t,h] * Wg[a,h]
  z[t,r]    = sum_a wh[a] * tanh(gh[t,a] + cv[r,a])
  alpha     = softmax_r(z)                       # output [B,T,R]
  cs[t,a]   = sum_h s_t[b,t,h] * Ws[a,h] + gh[t,a]
  z_ext[t]  = sum_a wh[a] * tanh(cs[t,a])
  beta      = softmax([z, z_ext])[..., -1]       # output [B,T,1]
  c_hat     = beta*s_t + (1-beta)*(alpha @ V)    # output [B,T,H]

Sharding: pure data parallel, batch dim 64 -> 8 cores x 8 batches.

Device dataflow per (b, t-tile of 128):
  PE   : content[t, a*49+r] = sum_c ghx[c,t]*REP[c,col]  (C=50: 49 ghT rows +
         a ones row carrying the flattened cv vector) -> PSUM, 5 matmuls N<=490.
  ACT  : tanh PSUM -> SBUF fp16 mega tile, layout col = a*50 + r (pitch 50 keeps
         every a-block 4-byte aligned for DVE 2x mode).
  DVE  : y = tanh * wh_rep (fp16 2x), then pairwise a-block add tree -> z.
  ACT  : exp over [z | z_ext]; DVE: sums/recip -> alpha, beta.
  PE   : alphasT = transpose((1-beta)*alpha); c' = alphasT.T @ V.
  DVE  : c_hat = (s_t * beta) + c'   (single fused scalar_tensor_tensor).
"""

import numpy as np

B, R, T, H, A = 64, 49, 512, 512, 49
NCORES = 8
BPC = B // NCORES            # batches per core
NTT = T // 128               # t-tiles per batch
PITCH = 50                   # a-block pitch in the tanh mega tile (49 data + 1 pad)
TTW = A * PITCH              # 2450 cols per t-tile in mega
CW = A * R                   # 2401 compact content cols
CHUNK = 490                  # content matmul free-dim chunk (10 a-blocks)

_CACHE = {}


def build_nc():
    import concourse.bass as bass
    import concourse.mybir as mybir
    from concourse import bacc, tile
    from concourse.tile_rust import add_dep_helper as tile_add_dep
    from contextlib import ExitStack

    dt = mybir.dt
    AF = mybir.ActivationFunctionType
    OP = mybir.AluOpType
    CONTENT_DT = dt.float16     # ghx / REP / content matmul ins
    TANH_DT = dt.float16         # tanh_out / wh_rep / tree

    nc = bacc.Bacc("TRN2", target_bir_lowering=False, debug=False)

    # ---- DRAM tensors (per-core shard; host ships partition-major layouts) ----
    hT = nc.dram_tensor("hT", [BPC, 128, 4, T], dt.float16, kind="ExternalInput")
    sT = nc.dram_tensor("sT", [BPC, 128, 4, T], dt.float16, kind="ExternalInput")
    st = nc.dram_tensor("st", [BPC, 128, NTT, H], dt.float16, kind="ExternalInput")
    v = nc.dram_tensor("v", [BPC, R, H], dt.float32, kind="ExternalInput")
    vT = nc.dram_tensor("vT", [BPC, 128, 4, R], dt.float16, kind="ExternalInput")
    wgT = nc.dram_tensor("wgT", [128, 4, A], dt.float16, kind="ExternalInput")
    wsT = nc.dram_tensor("wsT", [128, 4, A], dt.float16, kind="ExternalInput")
    wvT = nc.dram_tensor("wvT", [128, 4, A], dt.float16, kind="ExternalInput")
    repc = nc.dram_tensor("repc", [A, CW], CONTENT_DT, kind="ExternalInput")
    cvs = nc.dram_tensor("cvs", [BPC, 1, CW], CONTENT_DT, kind="Internal")
    whrep = nc.dram_tensor("whrep", [128, TTW], TANH_DT, kind="ExternalInput")
    wh49 = nc.dram_tensor("wh49", [128, NTT * A], TANH_DT, kind="ExternalInput")
    ident = nc.dram_tensor("ident", [128, 128], dt.float32, kind="ExternalInput")
    ones1 = nc.dram_tensor("ones1", [1, T], CONTENT_DT, kind="ExternalInput")

    chat_d = nc.dram_tensor("chat", [BPC, NTT, 128, H], dt.float32, kind="ExternalOutput")
    alpha_d = nc.dram_tensor("alpha", [BPC, 128, NTT, R], dt.float32, kind="ExternalOutput")
    beta_d = nc.dram_tensor("beta", [BPC, 128, NTT], dt.float32, kind="ExternalOutput")

    ctx = ExitStack()
    with tile.TileContext(nc) as tc, ctx:
        pool = ctx.enter_context(tc.tile_pool(name="main", bufs=1))
        cpool = ctx.enter_context(tc.tile_pool(name="const", bufs=1))
        ppool = ctx.enter_context(tc.tile_pool(name="ps", space="PSUM", bufs=1))

        # ---- constants (loaded once, distinct tags so nothing aliases) ----
        whrep_sb = cpool.tile([128, TTW], TANH_DT, tag="whrep")
        nc.gpsimd.dma_start(whrep_sb[:], whrep[:])
        wh49_sb = cpool.tile([128, NTT * A], TANH_DT, tag="wh49")
        nc.gpsimd.dma_start(wh49_sb[:], wh49[:])
        ident_sb = cpool.tile([128, 128], dt.float32, tag="ident")
        nc.gpsimd.dma_start(ident_sb[:], ident[:])
        wgT_sb = cpool.tile([128, 4 * A], dt.float16, tag="wgT")
        nc.gpsimd.dma_start(wgT_sb[:].rearrange("p (c a) -> p c a", a=A), wgT[:])
        wsT_sb = cpool.tile([128, 4 * A], dt.float16, tag="wsT")
        nc.gpsimd.dma_start(wsT_sb[:].rearrange("p (c a) -> p c a", a=A), wsT[:])
        wvT_sb = cpool.tile([128, 4 * A], dt.float16, tag="wvT")
        nc.gpsimd.dma_start(wvT_sb[:].rearrange("p (c a) -> p c a", a=A), wvT[:])
        # two parity REP tiles: rows 0..48 constant, row 49 = per-batch cv flat
        reps = []
        for par in range(2):
            rt = cpool.tile([A + 1, CW], CONTENT_DT, name=f"rep{par}", tag=f"rep{par}")
            nc.gpsimd.dma_start(rt[0:A, :], repc[:])
            reps.append(rt)

        def stage_A(b):
            """inputs, cvT->REP row, ghT/ghx, cs+tanh, content+tanh. PE/ACT/DMA."""
            rep_sb = reps[b % 2]
            vT_sb = pool.tile([128, 4 * R], dt.float16, tag="vT", bufs=3, name=f"vT{b}")
            nc.sync.dma_start(vT_sb[:].rearrange("p (c r) -> p c r", r=R), vT[b])
            v_sb = pool.tile([R, H], dt.float32, tag="v", bufs=3, name=f"v{b}")
            nc.sync.dma_start(v_sb[:], v[b])
            hT_sb = pool.tile([128, 4 * T], dt.float16, tag="hT", bufs=3, name=f"hT{b}")
            for c in range(4):
                nc.sync.dma_start(hT_sb[:, c * T:(c + 1) * T], hT[b, :, c])
            sT_sb = pool.tile([128, 4 * T], dt.float16, tag="sT", bufs=3, name=f"sT{b}")
            nc.sync.dma_start(sT_sb[:].rearrange("p (c t) -> p c t", t=T), sT[b])
            st_sb = pool.tile([128, NTT * H], dt.float16, tag="st", bufs=3, name=f"st{b}")
            nc.gpsimd.dma_start(st_sb[:].rearrange("p (c t) -> p c t", t=H), st[b])

            cvT_ps = ppool.tile([A, R], dt.float32, tag="ghTcs", bufs=1, name=f"cvT{b}")
            for c in range(4):
                nc.tensor.matmul(
                    out=cvT_ps[:],
                    lhsT=wvT_sb[:, c * A:(c + 1) * A],
                    rhs=vT_sb[:, c * R:(c + 1) * R],
                    start=(c == 0), stop=(c == 3),
                )
            cvf_sb = pool.tile([A, R], CONTENT_DT, tag="cvf", bufs=3, name=f"cvf{b}")
            nc.scalar.add(out=cvf_sb[:], in_=cvT_ps[:], add=0.0)
            wdma = nc.sync.dma_start(
                cvs[b].rearrange("one (a r) -> (one a) r", r=R), cvf_sb[:]
            )
            rdma = nc.sync.dma_start(rep_sb[A:A + 1, :], cvs[b])
            tile_add_dep(rdma.ins, wdma.ins, reason="cv flatten RAW through dram")

            ghT_ps = ppool.tile([A, T], dt.float32, tag="ghTcs", bufs=1, name=f"ghT{b}")
            for c in range(4):
                nc.tensor.matmul(
                    out=ghT_ps[:],
                    lhsT=wgT_sb[:, c * A:(c + 1) * A],
                    rhs=hT_sb[:, c * T:(c + 1) * T],
                    start=(c == 0), stop=(c == 3),
                )
            ghx_sb = pool.tile([A + 1, T], CONTENT_DT, tag="ghx", bufs=3, name=f"ghx{b}")
            nc.scalar.add(out=ghx_sb[0:A, :], in_=ghT_ps[:], add=0.0)
            nc.sync.dma_start(ghx_sb[A:A + 1, :], ones1[:])

            mega = pool.tile([128, NTT * TTW], TANH_DT, tag="mega", bufs=3, name=f"mega{b}")
            mv = mega[:].rearrange("p (t a r) -> p t a r", a=A, r=PITCH)
            nc.gpsimd.memset(mv[:, :, :, 49], 0.0)
            tcs = pool.tile([128, NTT * A], TANH_DT, tag="tcs", bufs=3, name=f"tcs{b}")
            for tt in range(NTT):
                cs_ps = ppool.tile([128, A], dt.float32, tag="ghTcs", bufs=1, name=f"cs{b}_{tt}")
                for c in range(4):
                    nc.tensor.matmul(
                        out=cs_ps[:],
                        lhsT=sT_sb[:, c * T + tt * 128: c * T + (tt + 1) * 128],
                        rhs=wsT_sb[:, c * A:(c + 1) * A],
                        start=(c == 0), stop=False,
                    )
                for c in range(4):
                    nc.tensor.matmul(
                        out=cs_ps[:],
                        lhsT=hT_sb[:, c * T + tt * 128: c * T + (tt + 1) * 128],
                        rhs=wgT_sb[:, c * A:(c + 1) * A],
                        start=False, stop=(c == 3),
                    )
                nc.scalar.activation(out=tcs[:, tt * A:(tt + 1) * A], in_=cs_ps[:], func=AF.Tanh)
                pa = ppool.tile([128, 1024], dt.float32, tag="cA", bufs=1, name=f"cA{b}_{tt}")
                pb = ppool.tile([128, 1024], dt.float32, tag="cB", bufs=1, name=f"cB{b}_{tt}")
                pc_ = ppool.tile([128, 512], dt.float32, tag="cC", bufs=1, name=f"cC{b}_{tt}")
                lhs = ghx_sb[:, tt * 128:(tt + 1) * 128]
                dests = [pa[:, 0:490], pa[:, 512:1002], pb[:, 0:490], pb[:, 512:1002],
                         pc_[:, 0:441]]
                for k in range(5):
                    n = CHUNK if k < 4 else CW - 4 * CHUNK
                    nc.tensor.matmul(
                        out=dests[k], lhsT=lhs,
                        rhs=rep_sb[:, k * CHUNK:k * CHUNK + n],
                        start=True, stop=True,
                    )
                base = tt * TTW
                for pt, nblk, off in ((pa, 10, 0), (pb, 10, 1000), (pc_, 9, 2000)):
                    if nblk == 10:
                        srcv = pt[:].rearrange("p (c x) -> p c x", x=512)[:, :, 0:490]
                        srcv = srcv.rearrange("p c (a r) -> p c a r", r=R)
                        dstv = mega[:, base + off: base + off + 1000]
                        dstv = dstv.rearrange("p (c a r) -> p c a r", c=2, r=PITCH)[:, :, :, 0:R]
                    else:
                        srcv = pt[:, 0:441].rearrange("p (a r) -> p a r", r=R)
                        dstv = mega[:, base + off: base + off + 450]
                        dstv = dstv.rearrange("p (a r) -> p a r", r=PITCH)[:, :, 0:R]
                    nc.scalar.activation(out=dstv, in_=srcv, func=AF.Tanh)
            return dict(st_sb=st_sb, v_sb=v_sb, mega=mega, mv=mv, tcs=tcs)

        def stage_B1(b, s, t0=0, t1=NTT):
            """wh multiply + pairwise tree + z_ext -> zm. DVE."""
            mega, mv, tcs = s["mega"], s["mv"], s["tcs"]
            nt = t1 - t0
            for tt in range(t0, t1):
                sl = mega[:, tt * TTW:(tt + 1) * TTW]
                nc.vector.tensor_tensor(out=sl, in0=sl, in1=whrep_sb[:], op=OP.mult)
            t1_ = pool.tile([128, NTT * 24 * PITCH], TANH_DT, tag="t1", name=f"t1_{b}_{t0}")
            t1v = t1_[:].rearrange("p (t a r) -> p t a r", a=24, r=PITCH)[:, t0:t1]
            nc.vector.tensor_tensor(out=t1v, in0=mv[:, t0:t1, 0:24, :], in1=mv[:, t0:t1, 25:49, :], op=OP.add)
            t2 = pool.tile([128, NTT * 12 * PITCH], TANH_DT, tag="t2", name=f"t2_{b}_{t0}")
            t2v = t2[:].rearrange("p (t a r) -> p t a r", a=12, r=PITCH)[:, t0:t1]
            nc.vector.tensor_tensor(out=t2v, in0=t1v[:, :, 0:12, :], in1=t1v[:, :, 12:24, :], op=OP.add)
            t3 = pool.tile([128, NTT * 6 * PITCH], TANH_DT, tag="t3", name=f"t3_{b}_{t0}")
            t3v = t3[:].rearrange("p (t a r) -> p t a r", a=6, r=PITCH)[:, t0:t1]
            nc.vector.tensor_tensor(out=t3v, in0=t2v[:, :, 0:6, :], in1=t2v[:, :, 6:12, :], op=OP.add)
            t4 = pool.tile([128, NTT * 3 * PITCH], TANH_DT, tag="t4", name=f"t4_{b}_{t0}")
            t4v = t4[:].rearrange("p (t a r) -> p t a r", a=3, r=PITCH)[:, t0:t1]
            nc.vector.tensor_tensor(out=t4v, in0=t3v[:, :, 0:3, :], in1=t3v[:, :, 3:6, :], op=OP.add)
            t5 = pool.tile([128, NTT * PITCH], TANH_DT, tag="t5", name=f"t5_{b}_{t0}")
            t5v = t5[:].rearrange("p (t r) -> p t r", r=PITCH)[:, t0:t1]
            nc.vector.tensor_tensor(out=t5v, in0=t4v[:, :, 0, :], in1=t4v[:, :, 1, :], op=OP.add)
            t6 = pool.tile([128, NTT * PITCH], TANH_DT, tag="t6", name=f"t6_{b}_{t0}")
            t6v = t6[:].rearrange("p (t r) -> p t r", r=PITCH)[:, t0:t1]
            nc.vector.tensor_tensor(out=t6v, in0=t5v[:], in1=t4v[:, :, 2, :], op=OP.add)
            if t0 == 0:
                s["zm"] = pool.tile([128, NTT * 64], dt.float32, tag="zm", bufs=2, name=f"zm{b}")
                s["zv"] = s["zm"][:].rearrange("p (t c) -> p t c", c=64)
            zv = s["zv"]
            nc.vector.tensor_tensor(
                out=zv[:, t0:t1, 0:R], in0=t6v[:, :, 0:R], in1=mv[:, t0:t1, 24, 0:R], op=OP.add
            )
            nc.vector.tensor_tensor(
                out=tcs[:, t0 * A:t1 * A], in0=tcs[:, t0 * A:t1 * A],
                in1=wh49_sb[:, 0:nt * A], op=OP.mult
            )
            ysv = tcs[:].rearrange("p (t a) -> p t a", a=A)[:, t0:t1]
            nc.vector.reduce_sum(out=zv[:, t0:t1, 49], in_=ysv, axis=mybir.AxisListType.X)

        def stage_B2(b, s, t0=0, t1=NTT):
            """exp + softmax + alpha/beta out. ACT(1) + small DVE."""
            zv = s["zv"]
            if t0 == 0:
                s["em"] = pool.tile([128, NTT * 64], dt.float32, tag="em", bufs=2, name=f"em{b}")
                s["ev"] = s["em"][:].rearrange("p (t c) -> p t c", c=64)
                s["s50"] = pool.tile([128, NTT], dt.float32, tag="s50", name=f"s50_{b}")
                s["s49"] = pool.tile([128, NTT], dt.float32, tag="s49", name=f"s49_{b}")
                s["r49"] = pool.tile([128, NTT], dt.float32, tag="r49", name=f"r49_{b}")
                s["r50"] = pool.tile([128, NTT], dt.float32, tag="r50", name=f"r50_{b}")
                s["beta_sb"] = pool.tile([128, NTT], dt.float32, tag="beta", bufs=2, name=f"beta{b}")
                s["omb"] = pool.tile([128, NTT], dt.float32, tag="omb", name=f"omb{b}")
                s["alpha_sb"] = pool.tile([128, NTT * R], dt.float32, tag="alpha", bufs=2, name=f"al{b}")
                s["av"] = s["alpha_sb"][:].rearrange("p (t r) -> p t r", r=R)
            ev, s50, s49 = s["ev"], s["s50"], s["s49"]
            r49, r50, beta_sb, omb, av = s["r49"], s["r50"], s["beta_sb"], s["omb"], s["av"]
            nc.scalar.activation(out=ev[:, t0:t1, 0:50], in_=zv[:, t0:t1, 0:50], func=AF.Exp)
            nc.vector.reduce_sum(out=s50[:, t0:t1], in_=ev[:, t0:t1, 0:50], axis=mybir.AxisListType.X)
            nc.vector.tensor_tensor(out=s49[:, t0:t1], in0=s50[:, t0:t1], in1=ev[:, t0:t1, 49], op=OP.subtract)
            nc.vector.reciprocal(out=r49[:, t0:t1], in_=s49[:, t0:t1])
            nc.vector.reciprocal(out=r50[:, t0:t1], in_=s50[:, t0:t1])
            nc.vector.tensor_tensor(out=beta_sb[:, t0:t1], in0=ev[:, t0:t1, 49], in1=r50[:, t0:t1], op=OP.mult)
            nc.vector.tensor_scalar(
                out=omb[:, t0:t1], in0=beta_sb[:, t0:t1], scalar1=-1.0, scalar2=1.0, op0=OP.mult, op1=OP.add
            )
            nc.sync.dma_start(beta_d[b][:, t0:t1], beta_sb[:, t0:t1])
            for tt in range(t0, t1):
                nc.vector.tensor_scalar(
                    out=av[:, tt, :], in0=ev[:, tt, 0:R],
                    scalar1=r49[:, tt:tt + 1], scalar2=None, op0=OP.mult,
                )
            nc.sync.dma_start(alpha_d[b][:, t0:t1], av[:, t0:t1])

        def stage_C(b, s, t0=0, t1=NTT):
            """c' = ((1-beta)*alpha) @ V; c_hat blend + store. PE + DVE."""
            av, omb, beta_sb = s["av"], s["omb"], s["beta_sb"]
            st_sb, v_sb = s["st_sb"], s["v_sb"]
            if t0 == 0:
                s["aT_sb"] = pool.tile([R, T], dt.float32, tag="aT", bufs=2, name=f"aT{b}")
                s["chat_sb"] = pool.tile([128, NTT * H], dt.float32, tag="chat", bufs=2, name=f"ch{b}")
            aT_sb, chat_sb = s["aT_sb"], s["chat_sb"]
            for tt in range(t0, t1):
                asc = pool.tile([128, R], dt.float32, tag="asc", bufs=3, name=f"asc{b}_{tt}")
                nc.vector.tensor_scalar(
                    out=asc[:], in0=av[:, tt, :],
                    scalar1=omb[:, tt:tt + 1], scalar2=None, op0=OP.mult,
                )
                aT_ps = ppool.tile([R, 128], dt.float32, tag="smallps", bufs=1, name=f"aT{b}_{tt}")
                nc.tensor.transpose(out=aT_ps[:], in_=asc[:], identity=ident_sb[:])
                nc.scalar.add(out=aT_sb[:, tt * 128:(tt + 1) * 128], in_=aT_ps[:], add=0.0)
            for tt in range(t0, t1):
                cps = ppool.tile([128, H], dt.float32, tag="cps", bufs=1, name=f"cps{b}_{tt}")
                nc.tensor.matmul(
                    out=cps[:], lhsT=aT_sb[:, tt * 128:(tt + 1) * 128], rhs=v_sb[:],
                    start=True, stop=True,
                )
                nc.vector.scalar_tensor_tensor(
                    out=chat_sb[:, tt * H:(tt + 1) * H], in0=st_sb[:, tt * H:(tt + 1) * H],
                    scalar=beta_sb[:, tt:tt + 1], in1=cps[:],
                    op0=OP.mult, op1=OP.add,
                )
                nc.sync.dma_start(chat_d[b, tt], chat_sb[:, tt * H:(tt + 1) * H])

        # Software pipeline: per iteration emit softmax(b-1), c_hat(b-1),
        # then content/tanh(b), then the DVE reduction(b). Keeps every
        # engine's in-order queue free of long cross-stage stalls.
        states = {}
        LAST = BPC - 1
        for it in range(BPC + 1):
            if it >= 1 and it - 1 != LAST:
                stage_B2(it - 1, states[it - 1])
            if it < BPC:
                states[it] = stage_A(it)
            if it >= 1 and it - 1 != LAST:
                stage_C(it - 1, states[it - 1])
            if it < BPC:
                if it == LAST:
                    stage_B1(it, states[it], 0, 2)
                    stage_B2(it, states[it], 0, 2)
                    stage_B1(it, states[it], 2, 4)
                else:
                    stage_B1(it, states[it])
            if it > 1:
                states.pop(it - 2, None)
        stage_C(LAST, states[LAST], 0, 2)
        stage_B2(LAST, states[LAST], 2, 4)
        stage_C(LAST, states[LAST], 2, 4)

    nc.compile()
    return nc


def host_prep(V, h_t, s_t, Wv, Wg, Ws, Wh):
    f32 = np.float32
    fp16 = np.float16
    bf16 = np.float16  # content path is fp16 end to end

    V = np.asarray(V, f32)
    h_t = np.asarray(h_t, f32)
    s_t = np.asarray(s_t, f32)
    wh = np.asarray(Wh, f32)[0]

    repc = np.zeros((A, CW), f32)
    for a in range(A):
        repc[a, a * R:(a + 1) * R] = 1.0
    whrep = np.zeros((TTW,), f32)
    whrep.reshape(A, PITCH)[:, :R] = wh[:, None]

    def wT(w):  # [A,H] -> [128, 4, A] partition-major chunks of w.T
        return np.ascontiguousarray(
            np.asarray(w, f32).T.reshape(4, 128, A).transpose(1, 0, 2)
        ).astype(bf16)

    consts = {
        "repc": np.ascontiguousarray(repc, dtype=bf16),
        "whrep": np.ascontiguousarray(np.broadcast_to(whrep, (128, TTW)), dtype=fp16),
        "wh49": np.ascontiguousarray(
            np.broadcast_to(np.tile(wh, NTT), (128, NTT * A)), dtype=fp16),
        "ident": np.eye(128, dtype=f32),
        "ones1": np.ones((1, T), dtype=bf16),
        "wgT": wT(Wg), "wsT": wT(Ws), "wvT": wT(Wv),
    }

    in_maps = []
    for core in range(NCORES):
        sl = slice(core * BPC, (core + 1) * BPC)
        hb, sb, vb = h_t[sl], s_t[sl], V[sl]
        m = dict(consts)
        # [BPC,T,H] -> transpose -> [BPC,H,T] -> [BPC,4,128,T] -> [BPC,128,4,T]
        m["hT"] = np.ascontiguousarray(
            hb.transpose(0, 2, 1).reshape(BPC, 4, 128, T).transpose(0, 2, 1, 3)
        ).astype(bf16)
        m["sT"] = np.ascontiguousarray(
            sb.transpose(0, 2, 1).reshape(BPC, 4, 128, T).transpose(0, 2, 1, 3)
        ).astype(bf16)
        m["st"] = np.ascontiguousarray(
            sb.reshape(BPC, NTT, 128, H).transpose(0, 2, 1, 3)
        ).astype(fp16)
        m["v"] = np.ascontiguousarray(vb)
        m["vT"] = np.ascontiguousarray(
            vb.transpose(0, 2, 1).reshape(BPC, 4, 128, R).transpose(0, 2, 1, 3)
        ).astype(bf16)
        in_maps.append(m)
    return in_maps


def gather(results):
    chat = np.concatenate([np.asarray(r["chat"]).reshape(BPC, T, H) for r in results], axis=0)
    alpha = np.concatenate(
        [np.asarray(r["alpha"]).transpose(0, 2, 1, 3).reshape(BPC, T, R) for r in results], axis=0
    )
    beta = np.concatenate(
        [np.asarray(r["beta"]).transpose(0, 2, 1).reshape(BPC, T, 1) for r in results], axis=0
    )
    return chat, alpha, beta


def kernel(V, h_t, s_t, Wv, Wg, Ws, Wh):
    from concourse.bass_utils import run_bass_kernel_spmd

    if "nc" not in _CACHE:
        _CACHE["nc"] = build_nc()
    nc = _CACHE["nc"]
    in_maps = host_prep(V, h_t, s_t, Wv, Wg, Ws, Wh)
    res = run_bass_kernel_spmd(nc, in_maps, core_ids=list(range(NCORES)))
    return gather(res.results)
